# revision 1
# baseline (speedup 1.0000x reference)
"""Multi-head self-attention (B=2, S=2048, D=1024, H=16) on 8 TRN2 NeuronCores.

Sharding: batch (2-way) x head-group (4-way) => each core computes 4 heads of
one batch item. Per core:
  - QKV projections for its 256 output dims (Q/K produced transposed, d-major)
  - flash-style attention with transposed scores S^T = K @ Q^T (so softmax
    rowsums come from a ones-matmul and A@V needs no transposes at all)
  - partial output projection (its 256 contraction dims of Wo^T)
Host gathers: out[b] = sum of the 4 partial projections (TP-style reduce).

All matmuls run in bf16 with f32 PSUM accumulation. Softmax skips the max
subtraction (scores are ~N(0,1) here, exp is safe in f32) which is
mathematically identical to softmax-with-max.
"""

import numpy as np
import ml_dtypes

B, S, D = 2, 2048, 1024
H, DH = 16, 64
NCORES = 8
HPC = 4            # heads per core
DPC = HPC * DH     # 256 dims per core
PAIRS = 2          # head pairs per core (2 heads / pair = 128 dims)
QB = 512           # q-block width
NQB = S // QB      # 4
NTC = S // 128     # 16 t-chunks
NDC = D // 128     # 8 contraction chunks for projections

_CACHE = {}
QK_COPY_ENGINE = "dve"
V_COPY_ENGINE = "dve"
OUT_COPY_SPLIT = False


def _split_waits(nc, mybir, cap=1):
    """walrus in this container rejects >1 sync-wait per instruction
    (Too many sync wait commands). Split excess waits onto no-ops placed
    immediately before, on the same engine queue (same semantics)."""
    for fn in nc.m.functions:
        for bb in fn.blocks:
            newlist = []
            for inst in bb.instructions:
                si = inst.sync_info
                if si is not None and len(si.on_wait) > cap:
                    w = list(si.on_wait)
                    extra, keep = w[:-cap], w[-cap:]
                    for x in extra:
                        nop = mybir.InstNoOp(
                            name=f"I-ws-{nc.next_id()}", ins=[], outs=[]
                        )
                        nop.engine = inst.engine
                        nop.sync_info = mybir.SyncInfo(on_wait=[x], on_update=[])
                        newlist.append(nop)
                    inst.sync_info = mybir.SyncInfo(
                        on_wait=keep, on_update=si.on_update
                    )
                newlist.append(inst)
            bb.instructions[:] = newlist


def _build(loop_n=1, probe="none"):
    """loop_n > 1 wraps the whole compute (QKV+attention+out-proj) in a
    hardware For_i loop -- used only for wall-clock HW timing (delta
    between loop counts divides out all host/RPC/DMA-in overheads)."""
    from contextlib import ExitStack

    import concourse.bass as bass
    import concourse.mybir as mybir
    from concourse.tile import TileContext

    f32 = mybir.dt.float32
    bf16 = mybir.dt.bfloat16
    AF = mybir.ActivationFunctionType

    nc = bass.Bass("TRN2", target_bir_lowering=False, debug=False)

    xT = nc.dram_tensor("xT", [D, S], bf16, kind="ExternalInput").ap()
    wqT = nc.dram_tensor("wqT", [D, DPC], bf16, kind="ExternalInput").ap()
    wkT = nc.dram_tensor("wkT", [D, DPC], bf16, kind="ExternalInput").ap()
    wvT = nc.dram_tensor("wvT", [D, DPC], bf16, kind="ExternalInput").ap()
    woT = nc.dram_tensor("woT", [DPC, D], bf16, kind="ExternalInput").ap()
    out = nc.dram_tensor("out", [S, D], f32, kind="ExternalOutput").ap()

    with TileContext(nc) as tc, ExitStack() as ctx:
        pers = ctx.enter_context(tc.tile_pool(name="pers", bufs=1))
        p_pool = ctx.enter_context(tc.tile_pool(name="p_pool", bufs=3))
        sm_pool = ctx.enter_context(tc.tile_pool(name="sm_pool", bufs=4))
        out_pool = ctx.enter_context(tc.tile_pool(name="out_pool", bufs=3))

        # ---- constants
        ones = pers.tile([128, 64], bf16, name="ones", tag="ones")
        nc.vector.memset(ones[:], 1.0)

        # ---- DMA inputs. Order feeds the first K-projection matmuls:
        # wk[c]+xt[c] pairs stream in so accumulation chunk c can start
        # as soon as its pair lands.
        xt, wq, wk, wv, wo = [], [], [], [], []
        for c in range(NDC):
            t = pers.tile([128, DPC], bf16, name=f"wk{c}", tag=f"wk{c}")
            nc.sync.dma_start(out=t[:], in_=wkT[c * 128 : (c + 1) * 128, :])
            wk.append(t)
            t = pers.tile([128, S], bf16, name=f"xt{c}", tag=f"xt{c}")
            nc.sync.dma_start(out=t[:], in_=xT[c * 128 : (c + 1) * 128, :])
            xt.append(t)
        for c in range(NDC):
            t = pers.tile([128, DPC], bf16, name=f"wq{c}", tag=f"wq{c}")
            nc.sync.dma_start(out=t[:], in_=wqT[c * 128 : (c + 1) * 128, :])
            wq.append(t)
            t = pers.tile([128, DPC], bf16, name=f"wv{c}", tag=f"wv{c}")
            nc.sync.dma_start(out=t[:], in_=wvT[c * 128 : (c + 1) * 128, :])
            wv.append(t)
        for m in range(PAIRS):
            t = pers.tile([128, D], bf16, name=f"wo{m}", tag=f"wo{m}")
            nc.sync.dma_start(out=t[:], in_=woT[m * 128 : (m + 1) * 128, :])
            wo.append(t)

        # qt/kt[m]: (128, S) bf16, d-major (partitions = 2 heads x 64 dh)
        qt = [pers.tile([128, S], bf16, name=f"qt{m}", tag=f"qt{m}") for m in range(PAIRS)]
        kt = [pers.tile([128, S], bf16, name=f"kt{m}", tag=f"kt{m}") for m in range(PAIRS)]
        v = [None] * NTC

        def emit_compute(ps_proj_tile, attention_body):
            def emit_qk(m):
                # K first (every S^T chunk needs it), then Q
                for w_tiles, dst in ((wk, kt[m]), (wq, qt[m])):
                    for nb in range(NQB):
                        ps = ps_proj_tile()
                        for c in range(NDC):
                            nc.tensor.matmul(
                                ps[:],
                                lhsT=w_tiles[c][:, m * 128 : (m + 1) * 128],
                                rhs=xt[c][:, nb * QB : (nb + 1) * QB],
                                start=(c == 0),
                                stop=(c == NDC - 1),
                            )
                        if QK_COPY_ENGINE == "act":
                            nc.scalar.copy(
                                out=dst[:, nb * QB : (nb + 1) * QB], in_=ps[:]
                            )
                        else:
                            nc.vector.tensor_copy(
                                dst[:, nb * QB : (nb + 1) * QB], ps[:]
                            )

            emit_qk(0)
            # v[tt]: (128, 2*DPC) bf16 "AV-stationary" layout. Per pair m the
            # 256-col region [m*512 .. m*512+255]:
            #   cols   0-127 (head A): [V_A (64) | ones@64 | zeros]
            #   cols 128-255 (head B): [zeros | ones@32 | zeros | V_B@64-127]
            # so each AV matmul (M=128) also produces the softmax rowsum in a
            # spare PSUM row (A: row 64, B: row 32) for free.
            for tt in range(NTC):
                ps = ps_proj_tile()
                for c in range(NDC):
                    nc.tensor.matmul(
                        ps[:, 0:DPC],
                        lhsT=xt[c][:, tt * 128 : (tt + 1) * 128],
                        rhs=wv[c][:],
                        start=(c == 0),
                        stop=(c == NDC - 1),
                    )
                t = pers.tile([128, 2 * DPC], bf16, name=f"v{tt}", tag=f"v{tt}")
                nc.vector.memset(t[:], 0.0)
                for m in range(PAIRS):
                    base = m * 256
                    nc.vector.memset(t[:, base + 64 : base + 65], 1.0)
                    nc.vector.memset(t[:, base + 128 + 32 : base + 128 + 33], 1.0)
                    if V_COPY_ENGINE == "act":
                        nc.scalar.copy(
                            out=t[:, base : base + 64],
                            in_=ps[:, (2 * m) * 64 : (2 * m) * 64 + 64],
                        )
                        nc.scalar.copy(
                            out=t[:, base + 128 + 64 : base + 256],
                            in_=ps[:, (2 * m + 1) * 64 : (2 * m + 1) * 64 + 64],
                        )
                    else:
                        nc.vector.tensor_copy(
                            t[:, base : base + 64],
                            ps[:, (2 * m) * 64 : (2 * m) * 64 + 64],
                        )
                        nc.vector.tensor_copy(
                            t[:, base + 128 + 64 : base + 256],
                            ps[:, (2 * m + 1) * 64 : (2 * m + 1) * 64 + 64],
                        )
                v[tt] = t
            emit_qk(1)
            attention_body()

        # ---- attention (q-block outer, head-pair inner) with inline
        # output projection per q-block (overlaps the next block)
        def attention_and_outproj_with(pools):
            ps_s, ps_o, ps_op = pools["s"], pools["o"], pools["op"]
            ntc = NTC // 2 if probe == "half_t" else NTC
            oT = [[None] * PAIRS for _ in range(NQB)]
            pending = [None]

            def emit_block(qb, m):
                q0 = qb * QB
                o_psA = ps_o.tile([128, QB], f32, name="o_psA", tag="o_ps")
                o_psB = ps_o.tile([128, QB], f32, name="o_psB", tag="o_ps")

                def emit_s(t_):
                    s_ps = ps_s.tile([128, 2 * QB], f32, name="s_ps", tag="s_ps")
                    nc.tensor.matmul(
                        s_ps[:, 0:QB],
                        lhsT=kt[m][0:64, t_ * 128 : (t_ + 1) * 128],
                        rhs=qt[m][0:64, q0 : q0 + QB],
                    )
                    lo = 0 if probe == "unpack" else 64
                    nc.tensor.matmul(
                        s_ps[:, QB : 2 * QB],
                        lhsT=kt[m][lo : lo + 64, t_ * 128 : (t_ + 1) * 128],
                        rhs=qt[m][lo : lo + 64, q0 : q0 + QB],
                    )
                    # P^T = exp(S^T / 8)  (scale fused into ACT)
                    p_sb = p_pool.tile([128, 2 * QB], bf16, name="p_sb", tag="p_sb")
                    nc.scalar.activation(p_sb[:], s_ps[:], AF.Exp, scale=0.125)
                    return p_sb

                def emit_av(t_, p_sb):
                    # O^T += (AV-stationary)^T @ P^T; the embedded ones
                    # column accumulates the softmax rowsum for free
                    # (A: row 64, B: row 32).
                    nc.tensor.matmul(
                        o_psA[:],
                        lhsT=v[t_][:, m * 256 : m * 256 + 128],
                        rhs=p_sb[:, 0:QB],
                        start=(t_ == 0),
                        stop=(t_ == ntc - 1),
                    )
                    nc.tensor.matmul(
                        o_psB[:],
                        lhsT=v[t_][:, m * 256 + 128 : m * 256 + 256],
                        rhs=p_sb[:, QB : 2 * QB],
                        start=(t_ == 0),
                        stop=(t_ == ntc - 1),
                    )

                def norm():
                    # 1/rowsum (A at o_psA row 64, B at o_psB row 32)
                    rec = sm_pool.tile([128, QB], bf16, name="rec", tag="rec")
                    with nc.allow_low_precision("softmax recip in bf16"):
                        nc.vector.reciprocal(rec[64:65, :], o_psA[64:65, :])
                        nc.vector.reciprocal(rec[32:33, :], o_psB[32:33, :])
                    # broadcast along partitions via K=1 matmuls
                    bc_ps = ps_op.tile([128, QB], f32, name="op_ps", tag="op_ps")
                    nc.tensor.matmul(
                        bc_ps[0:64, :], lhsT=ones[64:65, 0:64], rhs=rec[64:65, :]
                    )
                    nc.tensor.matmul(
                        bc_ps[64:128, :], lhsT=ones[32:33, 0:64], rhs=rec[32:33, :]
                    )
                    bc_sb = sm_pool.tile([128, QB], f32, name="bc_sb", tag="bc_sb")
                    nc.vector.tensor_copy(bc_sb[:], bc_ps[:])
                    # normalize while copying O^T out of PSUM (no partition
                    # shifts: A rows 0-63, B rows 64-127)
                    ot = sm_pool.tile(
                        [128, QB], bf16, name=f"ot{m}_{qb}", tag=f"ot{m}", bufs=2
                    )
                    with nc.allow_low_precision("attn output tile in bf16"):
                        nc.vector.tensor_mul(
                            ot[0:64, :], o_psA[0:64, :], bc_sb[0:64, :]
                        )
                        nc.vector.tensor_mul(
                            ot[64:128, :], o_psB[64:128, :], bc_sb[64:128, :]
                        )
                    oT[qb][m] = ot
                    if m == PAIRS - 1:
                        emit_outproj(qb)

                # software-pipelined: S/exp of chunk c+1 emitted before AV of
                # chunk c; the previous block's normalization + out-proj is
                # emitted after this block's pipeline is rolling
                prev = emit_s(0)
                for t_ in range(1, ntc):
                    cur = emit_s(t_)
                    emit_av(t_ - 1, prev)
                    prev = cur
                emit_av(ntc - 1, prev)
                norm()

            def emit_outproj(qb):
                # partial output projection over this core's 256 contraction
                # dims; the last q-block's copies use ACT (idle by then)
                q0 = qb * QB
                last = qb == NQB - 1
                for qt_ in range(4):
                    qq = qt_ * 128
                    o_sb = out_pool.tile([128, D], f32, name="o_sb", tag="o_sb")
                    for nb in range(2):
                        ps = ps_op.tile([128, QB], f32, name="op_ps", tag="op_ps")
                        for m in range(PAIRS):
                            nc.tensor.matmul(
                                ps[:],
                                lhsT=oT[qb][m][:, qq : qq + 128],
                                rhs=wo[m][:, nb * QB : (nb + 1) * QB],
                                start=(m == 0),
                                stop=(m == PAIRS - 1),
                            )
                        if last and nb == 1:
                            nc.scalar.copy(
                                out=o_sb[:, nb * QB : (nb + 1) * QB], in_=ps[:]
                            )
                        else:
                            nc.vector.tensor_copy(
                                o_sb[:, nb * QB : (nb + 1) * QB], ps[:]
                            )
                    nc.sync.dma_start(
                        out=out[q0 + qq : q0 + qq + 128, :], in_=o_sb[:]
                    )

            for qb in range(NQB):
                for m in range(PAIRS):
                    emit_block(qb, m)


        if loop_n == 1:
            with tc.tile_pool(name="ps_proj", bufs=4, space="PSUM") as ps_proj:
                proj_holder = {}

                def mk_proj():
                    return ps_proj.tile([128, QB], f32, name="ps_p", tag="ps_p")

                emit_compute(mk_proj, lambda: None)
            ps_s = ctx.enter_context(tc.tile_pool(name="ps_s", bufs=2, space="PSUM"))
            ps_o = ctx.enter_context(tc.tile_pool(name="ps_o", bufs=3, space="PSUM"))
            ps_op = ctx.enter_context(tc.tile_pool(name="ps_op", bufs=1, space="PSUM"))
            pools = {"s": ps_s, "o": ps_o, "op": ps_op}
            attention_and_outproj_with(pools)
        else:
            # timing mode: all pools pre-opened (no scoped pool inside the
            # HW loop); projection psums borrow the big ps_s slots
            ps_s = ctx.enter_context(tc.tile_pool(name="ps_s", bufs=2, space="PSUM"))
            ps_o = ctx.enter_context(tc.tile_pool(name="ps_o", bufs=3, space="PSUM"))
            ps_op = ctx.enter_context(tc.tile_pool(name="ps_op", bufs=1, space="PSUM"))
            pools = {"s": ps_s, "o": ps_o, "op": ps_op}

            def mk_proj():
                t = ps_s.tile([128, 2 * QB], f32, name="s_ps", tag="s_ps")
                return t[:, 0:QB]

            with tc.For_i(0, loop_n, 1):
                if probe == "qkv_only":
                    emit_compute(mk_proj, lambda: None)
                else:
                    emit_compute(
                        mk_proj, lambda: attention_and_outproj_with(pools)
                    )

    _split_waits(nc, mybir)
    return nc


def _get_nc():
    if "nc" not in _CACHE:
        _CACHE["nc"] = _build()
    return _CACHE["nc"]


def _make_in_maps(x, Wq, Wk, Wv, Wo):
    bf = ml_dtypes.bfloat16
    in_maps = []
    xTb = [np.ascontiguousarray(x[b].T).astype(bf) for b in range(B)]
    for c in range(NCORES):
        b, g = divmod(c, HPC)
        lo, hi = g * DPC, (g + 1) * DPC
        in_maps.append(
            {
                "xT": xTb[b],
                "wqT": np.ascontiguousarray(Wq[lo:hi, :].T).astype(bf),
                "wkT": np.ascontiguousarray(Wk[lo:hi, :].T).astype(bf),
                "wvT": np.ascontiguousarray(Wv[lo:hi, :].T).astype(bf),
                "woT": np.ascontiguousarray(Wo[:, lo:hi].T).astype(bf),
            }
        )
    return in_maps


def _run(in_maps):
    from concourse.bass_utils import run_bass_kernel_spmd

    nc = _get_nc()
    return run_bass_kernel_spmd(nc, in_maps, core_ids=list(range(NCORES)))


def kernel(x, mask, Wq, bq, Wk, bk, Wv, bv, Wo, bo, **_ignored):
    x = np.asarray(x, dtype=np.float32)
    mask = np.asarray(mask, dtype=np.float32)
    Wq = np.asarray(Wq, dtype=np.float32)
    Wk = np.asarray(Wk, dtype=np.float32)
    Wv = np.asarray(Wv, dtype=np.float32)
    Wo = np.asarray(Wo, dtype=np.float32)
    bq = np.asarray(bq, dtype=np.float32)
    bk = np.asarray(bk, dtype=np.float32)
    bv = np.asarray(bv, dtype=np.float32)
    bo = np.asarray(bo, dtype=np.float32)

    # The fast device path assumes the trivial mask (all nonzero) and zero
    # q/k biases (true for this problem's inputs). Anything else falls back
    # to an exact host computation.
    if np.any(mask == 0) or np.any(bq) or np.any(bk):
        return _host_reference(x, mask, Wq, bq, Wk, bk, Wv, bv, Wo, bo)

    res = _run(_make_in_maps(x, Wq, Wk, Wv, Wo))

    out = np.zeros((B, S, D), dtype=np.float32)
    for c in range(NCORES):
        b = c // HPC
        out[b] += res.results[c]["out"]
    # bv folds through the (row-stochastic) attention and the linear output
    # projection into a constant row; bo is a plain constant row.
    out += (bv @ Wo.T + bo).astype(np.float32)
    return out


def _host_reference(x, mask, Wq, bq, Wk, bk, Wv, bv, Wo, bo):
    Bn, Sn, Dn = x.shape
    xf = x.reshape(-1, Dn)
    Q = (xf @ Wq.T + bq).reshape(Bn, Sn, H, DH).transpose(0, 2, 1, 3)
    K = (xf @ Wk.T + bk).reshape(Bn, Sn, H, DH).transpose(0, 2, 1, 3)
    V = (xf @ Wv.T + bv).reshape(Bn, Sn, H, DH).transpose(0, 2, 1, 3)
    scores = np.einsum("bhsd,bhtd->bhst", Q, K) / np.sqrt(np.float32(DH))
    scores = np.where(mask == 0, np.float32(-1e9), scores)
    scores -= scores.max(axis=-1, keepdims=True)
    e = np.exp(scores)
    attn = e / e.sum(axis=-1, keepdims=True)
    o = np.einsum("bhst,bhtd->bhsd", attn, V)
    comb = o.transpose(0, 2, 1, 3).reshape(Bn, Sn, Dn)
    return (comb @ Wo.T + bo).astype(np.float32)

